# revision 69
# baseline (speedup 1.0000x reference)
"""Multi-head attention forward (B=8, S=1024, H=16, D=64) on 8 TRN2 NeuronCores.

Sharding: pure data-parallel over batch - core b computes batch element b
end-to-end (QKV projections + 16-head attention). Zero collectives.

Per-core dataflow (bf16 matmuls, fp32 PSUM accumulation):
  phase 0: x loads on the HWDGE queue (f32) + DVE cast to bf16 + PE-transpose
    to x^T layout; weight cast-loads ride the gpsimd (SWDGE) queue, pair-0
    column slices first so the first exp isn't gated on full weight loads.
  pair loop, slot-scheduled: per jt-slot one scores group (two heads packed
    on disjoint PE row-groups) + its exp on ScalarE; the exp-wait gaps are
    filled from a deque of small chunks: ctx matmuls of pair p-1, Q/K
    projection chain steps of pair p+1, V-projection steps (natural layout,
    no per-head transposes - lhsT = x_to^T tile, rhs = Wv), and the
    transpose-out/normalize work. This keeps the PE matmul stream dense
    (HAM warm) while ScalarE streams exps continuously.
  ctx'^T[65,i] = sum_jt [V_h | ones]_jt.T @ Et_jt (row 64 = softmax
    denominator, exact: probs sum to 1 so normalize(P_u@(V+bv)) == ctx+bv).
  Out stage: PE-transpose ctx'^T per s-tile into 68-el-aligned PSUM slots,
    ONE reciprocal + ONE broadcast tensor_tensor multiply per head.
"""

import numpy as np
from collections import deque
from contextlib import ExitStack

import concourse.bass as bass
import concourse.mybir as mybir
import concourse.tile as tile
from concourse import bacc
from concourse.masks import make_identity
from concourse.bass_utils import run_bass_kernel_spmd

B, S, H, D = 8, 1024, 16, 64
W = H * D  # 1024
P = 128
N_CORES = 8
F32 = mybir.dt.float32
BF16 = mybir.dt.bfloat16
AF = mybir.ActivationFunctionType
ALU = mybir.AluOpType

ST = S // P   # 8 s-tiles
KT_ = W // P  # 8 contraction tiles
IH = 2        # 512-wide halves of the moving dim
HD1 = D + 1   # 65: V strip width per head (V | ones)
HD2 = D + 2   # 66: padded strip stride in SBUF (4B aligned)
PO = D + 4    # 68: padded transpose-out slot (8B aligned in PSUM)
NP = H // 2   # 8 head pairs


def build_kernel(dbg=False):
    nc = bacc.Bacc(trn_type="TRN2", target_bir_lowering=False, debug=False,
                   num_devices=N_CORES)

    xf_ext = nc.dram_tensor("from_tensor", [S, W], F32, kind="ExternalInput").ap()
    xt_ext = nc.dram_tensor("to_tensor", [S, W], F32, kind="ExternalInput").ap()
    wq_ext = nc.dram_tensor("Wq", [W, W], F32, kind="ExternalInput").ap()
    bq_ext = nc.dram_tensor("bq", [W], F32, kind="ExternalInput").ap()
    wk_ext = nc.dram_tensor("Wk", [W, W], F32, kind="ExternalInput").ap()
    bk_ext = nc.dram_tensor("bk", [W], F32, kind="ExternalInput").ap()
    wv_ext = nc.dram_tensor("Wv", [W, W], F32, kind="ExternalInput").ap()
    bv_ext = nc.dram_tensor("bv", [W], F32, kind="ExternalInput").ap()
    out_ext = nc.dram_tensor("out", [S, W], F32, kind="ExternalOutput").ap()

    with tile.TileContext(nc) as tc, ExitStack() as top:
        const = top.enter_context(tc.tile_pool(name="const", bufs=1))
        big = top.enter_context(tc.tile_pool(name="big", bufs=1))

        ident = const.tile([P, P], BF16, tag="ident")
        make_identity(nc, ident[:])
        bq_sb = const.tile([P, KT_], F32, tag="bq")
        nc.gpsimd.dma_start(bq_sb[:], bq_ext.rearrange("(t p) -> p t", p=P))
        bk_sb = const.tile([P, KT_], F32, tag="bk")
        nc.gpsimd.dma_start(bk_sb[:], bk_ext.rearrange("(t p) -> p t", p=P))
        # bv replicated across partitions (varies along the free dim of
        # natural-layout V, so a per-partition scalar add can't apply it)
        bvrep = const.tile([P, W], F32, tag="bvrep")
        nc.gpsimd.dma_start(bvrep[:], bv_ext.partition_broadcast(P))

        # xT_all[p, kt*S + s] = x[s, kt*128+p]
        xTf_all = big.tile([P, KT_ * S], BF16, tag="xTf")
        xTt_all = big.tile([P, KT_ * S], BF16, tag="xTt")
        # w_all[p, kt*W + f] = Wx[kt*128+p, f]
        wq_all = big.tile([P, KT_ * W], BF16, tag="wq")
        wk_all = big.tile([P, KT_ * W], BF16, tag="wk")
        wv_all = big.tile([P, KT_ * W], BF16, tag="wv")
        # V natural, padded strips: vpad[p, ((jt*H + h)*HD2 + c)]
        vpad = big.tile([P, ST * H * HD2], BF16, tag="vpad")
        nc.vector.memset(
            vpad.rearrange("p (t h c) -> p t h c", h=H, c=HD2)[:, :, :, D:HD1],
            1.0)

        if dbg:
            d_vpad = nc.dram_tensor("d_vpad", [P, ST * H * HD2], BF16,
                                    kind="ExternalOutput").ap()

        def v_chunks(pool):
            """V in natural layout: 16 chains of 8 MMs (lhsT = x_to^T tile,
            rhs = Wv half); bias added and scattered into vpad strips on the
            way out of PSUM."""
            vp4 = vpad.rearrange("p (t h c) -> p t h c", h=H, c=HD2)
            bv4 = bvrep.rearrange("p (h c) -> p h c", c=D)
            state = {}

            def mk(st_, half, kt):
                def go():
                    if kt == 0:
                        state[0] = pool.tile([P, 512], F32, tag="proj",
                                             name="pv")
                    nc.tensor.matmul(
                        state[0][:],
                        lhsT=xTt_all[:, kt * S + st_ * P:
                                     kt * S + st_ * P + P],
                        rhs=wv_all[:, kt * W + half * 512:
                                   kt * W + (half + 1) * 512],
                        start=(kt == 0), stop=(kt == KT_ - 1))
                    if kt == KT_ - 1:
                        nc.vector.tensor_tensor(
                            vp4[:, st_, half * 8:(half + 1) * 8, 0:D],
                            state[0].rearrange("p (h c) -> p h c", c=D),
                            bv4[:, half * 8:(half + 1) * 8, :],
                            ALU.add)
                return go
            return [mk(st_, half, kt)
                    for st_ in range(ST) for half in range(IH)
                    for kt in range(KT_)]

        # projection PSUM pool lives across phase 0 and the pair loop so
        # pair-0 projections can start inside phase 0
        ps_proj = top.enter_context(
            tc.tile_pool(name="ps_proj", bufs=2, space="PSUM"))

        def proj_chunks(dstT, w_all, xT_all, b_sb, mt):
            """Projection chain as 16 chunks, one PSUM accumulator live
            at a time (ih-sequential)."""
            state = {}

            def mk(ih, kt):
                def go():
                    if kt == 0:
                        state[ih] = ps_proj.tile([P, 512], F32,
                                                 tag="proj", name="prj")
                    nc.tensor.matmul(
                        state[ih][:],
                        lhsT=w_all[:, kt * W + mt * P: kt * W + mt * P + P],
                        rhs=xT_all[:, kt * S + ih * 512:
                                   kt * S + (ih + 1) * 512],
                        start=(kt == 0), stop=(kt == KT_ - 1))
                    if kt == KT_ - 1:
                        nc.vector.tensor_scalar_add(
                            dstT[:, ih * 512:(ih + 1) * 512],
                            state[ih][:], b_sb[:, mt:mt + 1])
                return go
            return [mk(ih, kt) for ih in range(IH) for kt in range(KT_)]

        # ---- phase 0: load + cast + PE-transpose inputs ----
        QK = {}
        QK[0] = (big.tile([P, S], BF16, tag="qt0", name="QTp0"),
                 big.tile([P, S], BF16, tag="kt0", name="KTp0"))
        with ExitStack() as ph0:
            xn_pool = ph0.enter_context(tc.tile_pool(name="xn", bufs=1))
            ps_t = ph0.enter_context(
                tc.tile_pool(name="ps_t", bufs=4, space="PSUM"))
            xt_nat = xn_pool.tile([P, KT_ * W], BF16, tag="xtn", name="xtn")
            xf_nat = xn_pool.tile([P, KT_ * W], BF16, tag="xfn", name="xfn")

            # ALL loads ride the single gpsimd (SWDGE) queue in priority
            # order - one busy queue receives the full HBM share under the
            # SDMA round-robin instead of splitting it with the bulk weight
            # loads. Casts happen inside the DMA, so the DVE stays idle.
            def load_chunk(x_nat, x_ext, ch):
                nc.gpsimd.dma_start(
                    x_nat.rearrange("p (t w) -> p t w", w=W)[
                        :, 2 * ch:2 * ch + 2, :],
                    x_ext.rearrange("(t p) w -> p t w", p=P)[
                        :, 2 * ch:2 * ch + 2, :])

            def transpose_chunk(x_nat, xT_all, ch):
                for wt in range(KT_):
                    pt = ps_t.tile([P, 256], BF16, tag="pt", bufs=4, name="pt")
                    for sl in range(2):
                        nc.tensor.transpose(
                            pt[:, sl * P:(sl + 1) * P],
                            x_nat[:, (2 * ch + sl) * W + wt * P:
                                  (2 * ch + sl) * W + wt * P + P],
                            ident[:])
                    nc.vector.tensor_copy(
                        xT_all[:, wt * S + ch * 256: wt * S + (ch + 1) * 256],
                        pt[:])

            def load_w_cols(dst, src, c0, c1):
                nc.gpsimd.dma_start(
                    dst.rearrange("p (t f) -> p t f", f=W)[:, :, c0:c1],
                    src.rearrange("(t p) f -> p t f", p=P)[:, :, c0:c1])

            # x_to chunks first (K and V both need it), then the pair-0/1
            # Wk/Wq column slices, then x_from, then Wv and the bulk.
            for ch in range(4):
                load_chunk(xt_nat, xt_ext, ch)
                transpose_chunk(xt_nat, xTt_all, ch)
            load_w_cols(wk_all, wk_ext, 0, 2 * P)
            load_w_cols(wq_all, wq_ext, 0, 2 * P)
            for c in proj_chunks(QK[0][1], wk_all, xTt_all, bk_sb, 0):
                c()
            for ch in range(4):
                load_chunk(xf_nat, xf_ext, ch)
                transpose_chunk(xf_nat, xTf_all, ch)
            load_w_cols(wv_all, wv_ext, 0, W)
            load_w_cols(wk_all, wk_ext, 2 * P, W)
            load_w_cols(wq_all, wq_ext, 2 * P, W)
            for c in proj_chunks(QK[0][0], wq_all, xTf_all, bq_sb, 0):
                c()
            vwork = v_chunks(ps_proj)

        # ---- pair loop ----
        with ExitStack() as ph2:
            pp_pool = ph2.enter_context(tc.tile_pool(name="pp", bufs=1))
            et_pool = ph2.enter_context(tc.tile_pool(name="et", bufs=34))
            sm_pool = ph2.enter_context(tc.tile_pool(name="sm", bufs=1))
            op_pool = ph2.enter_context(tc.tile_pool(name="op", bufs=1))
            ps_s = ph2.enter_context(
                tc.tile_pool(name="ps_s", bufs=2, space="PSUM"))
            ps_c = ph2.enter_context(
                tc.tile_pool(name="ps_c", bufs=1, space="PSUM"))

            fillers = deque()

            def drain(n):
                for _ in range(n):
                    if not fillers:
                        return
                    fillers.popleft()()

            def ctx_chunks(hp, Et, out_p):
                """ctx' chains + transpose-out + batched normalize + DMA for
                pair hp, as a list of small chunks."""
                mt = hp
                op4 = out_p.rearrange("p (t g c) -> p t g c", g=2, c=D)
                state = {}
                chunks = []

                def mk_mm(hh, ih, jt):
                    def go():
                        if ih == 0 and jt == 0:
                            state[hh] = ps_c.tile([HD1, S], F32, tag="pcc",
                                                  name="pcc")
                        h = 2 * hp + hh
                        nc.tensor.matmul(
                            state[hh][:, ih * 512:(ih + 1) * 512],
                            lhsT=vpad[:, (jt * H + h) * HD2:
                                      (jt * H + h) * HD2 + HD1],
                            rhs=Et[jt][hh][:, ih * 512:(ih + 1) * 512],
                            start=(jt == 0), stop=(jt == ST - 1))
                    return go

                def mk_copy(hh):
                    def go():
                        ctxb = sm_pool.tile([HD1, S], BF16, tag="ctxb",
                                            bufs=3, name="ctxb")
                        nc.vector.tensor_copy(ctxb[:], state[hh][:])
                        state[(hh, "b")] = ctxb
                    return go

                def mk_po(hh, it0):
                    def go():
                        if it0 == 0:
                            state[(hh, "po")] = ps_proj.tile(
                                [P, ST * PO], BF16, tag="proj", name="po")
                        po = state[(hh, "po")]
                        ctxb = state[(hh, "b")]
                        for it in (it0, it0 + 1, it0 + 2, it0 + 3):
                            nc.tensor.transpose(
                                po[:, it * PO: it * PO + HD1],
                                ctxb[:, it * P:(it + 1) * P],
                                ident[0:HD1, 0:HD1])
                    return go

                def mk_norm(hh):
                    def go():
                        po3 = state[(hh, "po")].rearrange(
                            "p (t c) -> p t c", c=PO)
                        rinv = sm_pool.tile([P, ST], F32, tag="rinv", bufs=2,
                                            name="rinv")
                        nc.vector.reciprocal(rinv[:], po3[:, :, D:HD1])
                        nc.vector.tensor_tensor(
                            op4[:, :, hh, :], po3[:, :, 0:D],
                            rinv.rearrange("p (t o) -> p t o",
                                           o=1).to_broadcast((P, ST, D)),
                            ALU.mult)
                    return go

                for hh in range(2):
                    for ih in range(IH):
                        for jt in range(ST):
                            chunks.append(mk_mm(hh, ih, jt))
                    chunks.append(mk_copy(hh))
                    chunks.append(mk_po(hh, 0))
                    chunks.append(mk_po(hh, 4))
                    chunks.append(mk_norm(hh))

                def mk_out():
                    def go():
                        nc.sync.dma_start(
                            out_ext.rearrange("(t p) (g c) -> p t g c",
                                              p=P, c=P)[:, :, mt, :],
                            out_p.rearrange("p (t c) -> p t c", c=P))
                    return go
                chunks.append(mk_out())
                return chunks

            # Front of a pair: scores + exp per (jt, head), double-buffered
            # scores PSUM; each scores group is emitted one step AHEAD of
            # the exp stream so it sits in the PE queue before the filler
            # chunks - ScalarE then never waits on the next group.
            def emit_front(hp, QTp, KTp, per_slot, on_exp=None):
                def emit_scores(jt, hh):
                    ho = hh * D
                    pss = ps_s.tile([P, S], F32, tag="pss", name="pss")
                    for ih in range(IH):
                        nc.tensor.matmul(
                            pss[:, ih * 512:(ih + 1) * 512],
                            lhsT=KTp[ho:ho + D, jt * P: jt * P + P],
                            rhs=QTp[ho:ho + D, ih * 512:(ih + 1) * 512],
                            start=True, stop=True)
                    return pss

                seq = [(jt, hh) for jt in range(ST) for hh in range(2)]
                pss_q = {seq[0]: emit_scores(*seq[0])}
                Et = {jt: [None, None] for jt in range(ST)}
                for idx, (jt, hh) in enumerate(seq):
                    if idx + 1 < len(seq):
                        pss_q[seq[idx + 1]] = emit_scores(*seq[idx + 1])
                    et = et_pool.tile([P, S], BF16, tag="et", name="et")
                    nc.scalar.activation(et[:], pss_q.pop((jt, hh))[:],
                                         AF.Exp, scale=0.125)
                    Et[jt][hh] = et
                    if on_exp is not None:
                        on_exp(jt, hh, et)
                    drain(per_slot)
                return {jt: tuple(v) for jt, v in Et.items()}

            # ---- schedule ----
            # V drains inside front(0)'s exp gaps (the PE would otherwise
            # idle there - no prior pair's ctx exists yet; Wv lands just
            # before front(0) begins)
            fillers.extend(vwork)
            Et_prev = None
            for hp in range(NP):
                # safety: anything left over from earlier rounds (notably
                # this pair's own projections) must be emitted before its
                # scores reference the QK tiles
                drain(len(fillers))
                # ctx of pair hp-1 and projections of pair hp+1 drain as
                # fillers inside front(hp)'s exp-wait gaps
                if Et_prev is not None:
                    out_p = op_pool.tile([P, S], F32, tag="outp", bufs=2,
                                         name="out_p")
                    fillers.extend(ctx_chunks(hp - 1, Et_prev, out_p))
                if hp + 1 < NP:
                    QK[hp + 1] = (
                        pp_pool.tile([P, S], BF16, tag="qt", bufs=2,
                                     name="QTp"),
                        pp_pool.tile([P, S], BF16, tag="kt", bufs=2,
                                     name="KTp"))
                    fillers.extend(proj_chunks(
                        QK[hp + 1][1], wk_all, xTt_all, bk_sb, hp + 1))
                    fillers.extend(proj_chunks(
                        QK[hp + 1][0], wq_all, xTf_all, bq_sb, hp + 1))
                on_exp = None
                if hp == NP - 1:
                    # stream the last pair's hh0 ctx chain into this front's
                    # exp gaps so the tail only has hh1 + out left
                    out_last = op_pool.tile([P, S], F32, tag="outp", bufs=2,
                                            name="out_p")
                    last_state = {}

                    def on_exp(jt, hh, et, hp=hp):
                        if hh != 0:
                            return

                        def go(jt=jt, et=et):
                            if jt == 0:
                                last_state[0] = ps_c.tile(
                                    [HD1, S], F32, tag="pcc", name="pcc")
                            h = 2 * hp
                            for ih in range(IH):
                                nc.tensor.matmul(
                                    last_state[0][:, ih * 512:(ih + 1) * 512],
                                    lhsT=vpad[:, (jt * H + h) * HD2:
                                              (jt * H + h) * HD2 + HD1],
                                    rhs=et[:, ih * 512:(ih + 1) * 512],
                                    start=(jt == 0), stop=(jt == ST - 1))
                        fillers.append(go)
                Et = emit_front(hp, *QK[hp],
                                per_slot=10 if hp == 0 else 5, on_exp=on_exp)
                del QK[hp]
                Et_prev = Et
            # drain every remaining filler BEFORE the final ctx block so
            # chunk chains never emit out of order
            while fillers:
                fillers.popleft()()
            if dbg:
                nc.sync.dma_start(d_vpad, vpad[:])
            # tail: finish the last pair - hh0 was accumulated during
            # front(7); emit its out-stage, then all of hh1
            op4l = out_last.rearrange("p (t g c) -> p t g c", g=2, c=D)
            for hh in range(2):
                if hh == 0:
                    pc = last_state[0]
                else:
                    pc = ps_c.tile([HD1, S], F32, tag="pcc", name="pcc")
                    h = 2 * (NP - 1) + 1
                    for jt in range(ST):
                        for ih in range(IH):
                            nc.tensor.matmul(
                                pc[:, ih * 512:(ih + 1) * 512],
                                lhsT=vpad[:, (jt * H + h) * HD2:
                                          (jt * H + h) * HD2 + HD1],
                                rhs=Et_prev[jt][1][:, ih * 512:(ih + 1) * 512],
                                start=(jt == 0), stop=(jt == ST - 1))
                ctxb = sm_pool.tile([HD1, S], BF16, tag="ctxb", bufs=3,
                                    name="ctxb")
                nc.vector.tensor_copy(ctxb[:], pc[:])
                po = ps_proj.tile([P, ST * PO], BF16, tag="proj", name="po")
                po3 = po.rearrange("p (t c) -> p t c", c=PO)
                for it in range(ST):
                    nc.tensor.transpose(
                        po[:, it * PO: it * PO + HD1],
                        ctxb[:, it * P:(it + 1) * P],
                        ident[0:HD1, 0:HD1])
                rinv = sm_pool.tile([P, ST], F32, tag="rinv", bufs=2,
                                    name="rinv")
                nc.vector.reciprocal(rinv[:], po3[:, :, D:HD1])
                nc.vector.tensor_tensor(
                    op4l[:, :, hh, :], po3[:, :, 0:D],
                    rinv.rearrange("p (t o) -> p t o", o=1).to_broadcast(
                        (P, ST, D)),
                    ALU.mult)
            nc.sync.dma_start(
                out_ext.rearrange("(t p) (g c) -> p t g c", p=P, c=P)[
                    :, :, NP - 1, :],
                out_last.rearrange("p (t c) -> p t c", c=P))

    nc.compile()
    return nc


def run(inputs, trace=False, trace_kwargs=None):
    """inputs: dict of full-shape np arrays as in reference.setup_inputs()."""
    nc = build_kernel()
    in_maps = []
    for b in range(N_CORES):
        in_maps.append({
            "from_tensor": np.ascontiguousarray(np.asarray(inputs["from_tensor"][b], dtype=np.float32)),
            "to_tensor": np.ascontiguousarray(np.asarray(inputs["to_tensor"][b], dtype=np.float32)),
            "Wq": np.asarray(inputs["Wq"], dtype=np.float32),
            "bq": np.asarray(inputs["bq"], dtype=np.float32),
            "Wk": np.asarray(inputs["Wk"], dtype=np.float32),
            "bk": np.asarray(inputs["bk"], dtype=np.float32),
            "Wv": np.asarray(inputs["Wv"], dtype=np.float32),
            "bv": np.asarray(inputs["bv"], dtype=np.float32),
        })
    res = run_bass_kernel_spmd(nc, in_maps, core_ids=list(range(N_CORES)),
                               trace=trace, **(trace_kwargs or {}))
    out = np.stack([np.asarray(res.results[b]["out"]) for b in range(N_CORES)],
                   axis=0).astype(np.float32)
    return out, res


def kernel(**inputs):
    out, _ = run(inputs, trace=False)
    return out
